# revision 20
# baseline (speedup 1.0000x reference)
"""Bass/Trainium2 kernel for a 2-layer GRU decoder.

Contract: kernel(**inputs) takes the FULL unsharded inputs (numpy arrays,
keys as in setup_inputs) and returns the full (out, h) pair, matching the
jax reference. Internally shards the batch across 8 NeuronCores.

Design (per core, Bs = 8 sequences):
  - hosts pre-transpose x and all weights so the device never transposes
  - per 64-step window: GI chunk matmuls (input-side gates, dense GEMM),
    then the two layer recurrences step-interleaved so each layer's gate
    (DVE/ACT) latency hides behind the other layer's 48 recurrent matmuls
  - weights stationary (bf16), h streamed as rhs; gates in [3H, Bs] layout
"""

import sys

sys.path.insert(0, "/opt/trn_rl_repo")

import numpy as np
import ml_dtypes

import concourse.bass as bass
import concourse.mybir as mybir
from concourse.tile import TileContext
from concourse.vector_clock import ScopedClock, VectorClock
from concourse.tile_scheduler import N_PROCS
from concourse.bass import ts
from concourse import bass_utils


def _install_ntff_hook_shim():
    """The container's antenv lacks axon_hooks; synthesize it so
    run_bass_kernel_spmd(trace=True) can NTFF-profile via the axon .so."""
    import types
    import antenv

    if "antenv.axon_hooks" in sys.modules:
        return
    mod = types.ModuleType("antenv.axon_hooks")
    _state = {"hook": None}
    mod.set_axon_ntff_profile_hook = lambda h: _state.__setitem__("hook", h)
    mod.get_axon_ntff_profile_hook = lambda: _state["hook"]
    sys.modules["antenv.axon_hooks"] = mod
    antenv.axon_hooks = mod
    try:
        from trn_agent_boot.trn_boot import _ntff_profile_via_ctypes
        mod.set_axon_ntff_profile_hook(
            _ntff_profile_via_ctypes("/opt/axon/libaxon_pjrt.so"))
    except Exception:
        pass


_install_ntff_hook_shim()

BF16 = mybir.dt.float16  # fp16: 10-bit mantissa, plenty of range here
F32 = mybir.dt.float32
AF = mybir.ActivationFunctionType

B, T_FULL, IN, H, L = 64, 512, 512, 512, 2
NCORES = 8
BS = B // NCORES          # 8 sequences per core
H3 = 3 * H                # 1536
KT = H // 128             # 4 contraction tiles
MT = H3 // 128            # 12 output tiles (r:0-3, z:4-7, n:8-11)


# --- workaround: this walrus build allows only one sem-wait per control
# instruction; split the TileContext exit drain into one drain per proc.
def _drain_and_barrier_split(self, tick_clock, wait_clock):
    g = tick_clock.global_clock
    for p in range(N_PROCS):
        if g[p] == 0:
            continue
        vec = [0] * N_PROCS
        vec[p] = g[p]
        d = self.nc.sync.drain()
        wait_clock.add_sem_waits(d.ins, ScopedClock({None: VectorClock(vec)}))
    self.nc.all_engine_barrier()
    assert self.sems is not None
    popped = self.nc._tile_sem_poison_stack.pop()
    assert popped is self._sem_poison
    self.nc.clear_and_free_semaphores(list(self.sems.allocated().values()))
    self.nc.all_engine_barrier()


TileContext._drain_and_barrier = _drain_and_barrier_split

_COMPUTE_ENGINES = {
    mybir.EngineType.PE,
    mybir.EngineType.Activation,
    mybir.EngineType.DVE,
    mybir.EngineType.Pool,
    mybir.EngineType.SP,
}


def _split_multi_waits(nc):
    """This walrus build allows one sem-wait per engine instruction; hoist
    extra waits onto single-wait NoOps inserted just before."""
    ctr = 0
    for f in nc.m.functions:
        for bb in f.blocks:
            il = bb.instructions
            i = 0
            while i < len(il):
                inst = il[i]
                si = inst.sync_info
                if (si is not None and len(si.on_wait) > 1
                        and inst.engine in _COMPUTE_ENGINES):
                    waits = list(si.on_wait)
                    nops = []
                    for w in waits[:-1]:
                        ctr += 1
                        nop = mybir.InstNoOp(
                            name=f"waitsplit-{ctr}", ins=[], outs=[])
                        nop.engine = inst.engine
                        nop.sync_info = mybir.SyncInfo(
                            on_wait=[w], on_update=[])
                        nops.append(nop)
                    inst.sync_info = mybir.SyncInfo(
                        on_wait=[waits[-1]], on_update=list(si.on_update))
                    il[i:i] = nops
                    i += len(nops)
                i += 1
    return nc


def build_gru_nc(t_steps: int, w_steps: int) -> bass.Bass:
    n_win = t_steps // w_steps
    assert n_win * w_steps == t_steps
    nw = w_steps * BS  # free-dim columns per window chunk

    nc = bass.Bass()

    # ---- DRAM I/O (host-prepped layouts) ----
    xT = nc.dram_tensor("xT", [128, KT, t_steps * BS], BF16, kind="ExternalInput")
    wih = [nc.dram_tensor(f"wih{l}", [128, KT, H3], BF16, kind="ExternalInput")
           for l in range(L)]
    whh = [nc.dram_tensor(f"whh{l}", [128, KT, H3], BF16, kind="ExternalInput")
           for l in range(L)]
    bgi = [nc.dram_tensor(f"bgi{l}", [128, MT], F32, kind="ExternalInput")
           for l in range(L)]
    # b_hh n-part as a [KT, 128] lhsT + one-hot selector: bias lands in PSUM
    # via a single K=4 matmul instead of a DVE add on the critical chain
    bnw = [nc.dram_tensor(f"bnw{l}", [KT, 128], BF16, kind="ExternalInput")
           for l in range(L)]
    sel = nc.dram_tensor("sel", [KT, KT * BS], BF16, kind="ExternalInput")
    ench = [nc.dram_tensor(f"ench{l}", [128, KT, BS], F32, kind="ExternalInput")
            for l in range(L)]
    # layer-1 hidden sequence (the decoder output), [p, j, b, t]
    out1 = nc.dram_tensor("out1", [128, KT, BS, t_steps], F32, kind="ExternalOutput")
    # layer-0 final hidden state
    h0f = nc.dram_tensor("h0f", [128, KT, BS], F32, kind="ExternalOutput")

    with TileContext(nc) as tc:
        with (
            tc.tile_pool(name="weights", bufs=1) as wpool,
            tc.tile_pool(name="state", bufs=1) as spool,
            tc.tile_pool(name="xchunk", bufs=1) as xpool,
            tc.tile_pool(name="tmp", bufs=4) as tpool,
            tc.tile_pool(name="gipsum", bufs=2, space="PSUM") as gps,
            tc.tile_pool(name="recpsum", bufs=1, space="PSUM") as rps,
            tc.tile_pool(name="dumpsum", bufs=1, space="PSUM") as dps,
        ):
            # ---- persistent SBUF tiles ----
            wih_sb = [wpool.tile([128, KT, H3], BF16, name=f"wih{l}") for l in range(L)]
            whh_sb = [wpool.tile([128, KT, H3], BF16, name=f"whh{l}") for l in range(L)]
            bgi_sb = [wpool.tile([128, MT], F32, name=f"bgi{l}") for l in range(L)]
            bnw_sb = [wpool.tile([KT, 128], BF16, name=f"bnw{l}") for l in range(L)]
            sel_sb = wpool.tile([KT, KT * BS], BF16, name="sel_sb")
            # GI double-buffer slots, [p, m, i, b]
            gi0_sl = [spool.tile([128, MT, w_steps, BS], BF16, name=f"gi0_{s}")
                      for s in range(2)]
            gi1_sl = [spool.tile([128, MT, w_steps, BS], BF16, name=f"gi1_{s}")
                      for s in range(2)]
            # hidden-state rings, [p, k, pos, b]; pos 0 = window-entry state
            h0_sl = [spool.tile([128, KT, w_steps + 1, BS], BF16, name=f"h0_{s}")
                     for s in range(2)]
            h1_sl = [spool.tile([128, KT, w_steps + 1, BS], BF16, name=f"h1_{s}")
                     for s in range(2)]
            # fp32 layer-1 output staging, [p, j, b, i] (t innermost for DMA)
            ho_sl = [spool.tile([128, KT, BS, w_steps], F32, name=f"ho_{s}")
                     for s in range(2)]
            xt_sl = [xpool.tile([128, KT, nw], BF16, name=f"xt_{s}")
                     for s in range(2)]
            # recurrence psum, split rz|n into separate banks so the rz
            # gates can start while the n-tiles are still multiplying
            ps_rz = [rps.tile([128, 8, BS], F32, name=f"psrz{l}", tag=f"psrz{l}")
                     for l in range(L)]
            ps_n = [rps.tile([128, KT, BS], F32, name=f"psn{l}", tag=f"psn{l}")
                    for l in range(L)]

            # ---- prologue: weights, biases, initial hidden states ----
            for l in range(L):
                nc.sync.dma_start(wih_sb[l][:], wih[l][:])
                nc.sync.dma_start(whh_sb[l][:], whh[l][:])
                nc.sync.dma_start(bgi_sb[l][:], bgi[l][:])
                nc.sync.dma_start(bnw_sb[l][:], bnw[l][:])
            nc.sync.dma_start(sel_sb[:], sel[:])
            ench_sb = [tpool.tile([128, KT, BS], F32, tag=f"ench{l}",
                                  name=f"ench_sb{l}") for l in range(L)]
            for l in range(L):
                nc.sync.dma_start(ench_sb[l][:], ench[l][:])
            nc.vector.tensor_copy(h0_sl[0][:, :, 0, :], ench_sb[0][:])
            nc.vector.tensor_copy(h1_sl[0][:, :, 0, :], ench_sb[1][:])

            SUB = 8  # GI1 sub-chunk length in steps
            dum_ps = dps.tile([128, KT * BS], F32, name="dum_ps")

            def dummy_mms(n):
                """Keep-warm matmuls into a scratch bank: the PE would
                otherwise idle during gate latency and HAM re-throttles it
                to 1.2 GHz, doubling every real matmul."""
                for _ in range(n):
                    nc.tensor.matmul(
                        dum_ps[:], lhsT=bnw_sb[0][:], rhs=sel_sb[:],
                        start=True, stop=True)

            def gi0_microops(wdst):
                """Per-instruction closures for GI0 of window wdst (from x)."""
                sl = wdst % 2
                ops = []
                for m in range(MT):
                    pg = gps.tile([128, w_steps, BS], F32, tag="gip0", bufs=1,
                                  name=f"gip0_{wdst}_{m}")
                    for k in range(KT):
                        ops.append(lambda m=m, k=k, pg=pg: nc.tensor.matmul(
                            pg[:], lhsT=wih_sb[0][:, k, ts(m, 128)],
                            rhs=xt_sl[sl][:, k, :],
                            start=(k == 0), stop=(k == KT - 1)))
                    ops.append(lambda m=m, pg=pg: nc.scalar.activation(
                        gi0_sl[sl][:, m], pg[:], AF.Identity,
                        bias=bgi_sb[0][:, m:m + 1]))
                return ops

            def gi1_subchunk_ops(w, q):
                """m-tile closures for GI1 window w, steps [SUB*q, SUB*(q+1))."""
                sl = w % 2
                ops = []
                for m in range(MT):
                    def one(m=m, q=q, sl=sl):
                        pg = gps.tile([128, SUB, BS], F32, tag="gip1",
                                      name=f"gip1_{w}_{q}_{m}")
                        for k in range(KT):
                            nc.tensor.matmul(
                                pg[:], lhsT=wih_sb[1][:, k, ts(m, 128)],
                                rhs=h1rhs(sl, k, q),
                                start=(k == 0), stop=(k == KT - 1))
                        nc.scalar.activation(
                            gi1_sl[sl][:, m, ts(q, SUB), :], pg[:], AF.Identity,
                            bias=bgi_sb[1][:, m:m + 1])
                    ops.append(one)
                return ops

            def h1rhs(sl, k, q):
                return h0_sl[sl][:, k, 1 + SUB * q: 1 + SUB * (q + 1), :]

            def step(l, h_slot, i, gi_slot, ho_slot):
                """One GRU cell step: h[pos i] -> h[pos i+1]."""
                prz, pn = ps_rz[l], ps_n[l]
                # r/z recurrent matmuls first -> their own psum bank
                for m in range(8):
                    for k in range(KT):
                        nc.tensor.matmul(
                            prz[:, m, :],
                            lhsT=whh_sb[l][:, k, ts(m, 128)],
                            rhs=h_slot[:, k, i, :],
                            start=(k == 0), stop=(k == KT - 1))
                # n-part: bias lands via one K=KT selector matmul, then accum
                nc.tensor.matmul(pn[:], lhsT=bnw_sb[l][:], rhs=sel_sb[:],
                                 start=True, stop=False, skip_group_check=True)
                for m in range(KT):
                    for k in range(KT):
                        nc.tensor.matmul(
                            pn[:, m, :],
                            lhsT=whh_sb[l][:, k, ts(8 + m, 128)],
                            rhs=h_slot[:, k, i, :],
                            start=False, stop=(k == KT - 1),
                            skip_group_check=True)
                gi_rz = gi_slot[:, 0:8, i, :]
                gi_n = gi_slot[:, 8:12, i, :]
                h_prev = h_slot[:, :, i, :]
                h_out = h_slot[:, :, i + 1, :]

                trz = tpool.tile([128, 8, BS], F32, tag="trz")
                nc.vector.tensor_add(trz[:], prz[:, 0:8, :], gi_rz)
                rzb = tpool.tile([128, 8, BS], BF16, tag="rzb")
                nc.scalar.activation(rzb[:], trz[:], AF.Sigmoid)
                # t1 = z*h_prev runs in parallel with the tanh path
                t1 = tpool.tile([128, KT, BS], BF16, tag="t1")
                nc.vector.tensor_mul(t1[:], rzb[:, 4:8, :], h_prev)
                u = tpool.tile([128, KT, BS], F32, tag="u")
                nc.vector.tensor_mul(u[:], pn[:], rzb[:, 0:4, :])
                wt = tpool.tile([128, KT, BS], F32, tag="wt")
                nc.vector.tensor_add(wt[:], u[:], gi_n)
                nt = tpool.tile([128, KT, BS], BF16, tag="nt")
                nc.scalar.activation(nt[:], wt[:], AF.Tanh)
                # a = (z-1)*n ; h' = z*h - a = z*h + (1-z)*n
                a = tpool.tile([128, KT, BS], BF16, tag="a")
                nc.vector.scalar_tensor_tensor(
                    a[:], rzb[:, 4:8, :], 1.0, nt[:],
                    op0=mybir.AluOpType.subtract, op1=mybir.AluOpType.mult)
                nc.vector.tensor_sub(h_out, t1[:], a[:])
                if ho_slot is not None:
                    # fp32 copy of h for the output stream (off the chain)
                    nc.gpsimd.tensor_sub(ho_slot[:, :, :, i], t1[:], a[:])

            # ---- main loop: window w runs L0 on [64w, 64w+64) and L1 one
            # window behind; GI0(w+1) and GI1(w) matmuls are spread between
            # steps to keep the PE busy (and HAM-warm) during gate latency.
            nc.sync.dma_start(xt_sl[0][:], xT[:, :, ts(0, nw)])
            for op in gi0_microops(0):
                op()
            for w in range(n_win + 1):
                s = w % 2
                sp = (w - 1) % 2
                gi0_q = []
                if w + 1 < n_win:
                    nc.sync.dma_start(xt_sl[(w + 1) % 2][:],
                                      xT[:, :, ts(w + 1, nw)])
                    gi0_q = gi0_microops(w + 1)
                gi1_q = []
                for i in range(w_steps):
                    if w < n_win:
                        step(0, h0_sl[s], i, gi0_sl[s], None)
                        if i % SUB == SUB - 1 and i != w_steps - 1:
                            gi1_q.extend(gi1_subchunk_ops(w, i // SUB))
                    if w >= 1:
                        step(1, h1_sl[sp], i, gi1_sl[sp], ho_sl[sp])
                    # filler: keep the PE streaming between steps
                    if gi0_q:
                        gi0_q.pop(0)()
                    for _ in range(2):
                        if gi1_q:
                            gi1_q.pop(0)()
                    dummy_mms(10)
                if w < n_win:
                    gi1_q.extend(gi1_subchunk_ops(w, w_steps // SUB - 1))
                    for op in gi1_q:
                        op()
                    for op in gi0_q:
                        op()
                    # carry window-exit state into the next ring slot
                    if w + 1 < n_win:
                        nc.vector.tensor_copy(h0_sl[(w + 1) % 2][:, :, 0, :],
                                              h0_sl[s][:, :, w_steps, :])
                if w >= 1:
                    nc.sync.dma_start(out1[:, :, :, ts(w - 1, w_steps)],
                                      ho_sl[sp][:])
                    if w < n_win:
                        nc.vector.tensor_copy(h1_sl[w % 2][:, :, 0, :],
                                              h1_sl[sp][:, :, w_steps, :])

            # layer-0 final hidden state -> fp32 -> DRAM
            h0fin = tpool.tile([128, KT, BS], F32, tag="h0fin")
            nc.vector.tensor_copy(h0fin[:], h0_sl[(n_win - 1) % 2][:, :, w_steps, :])
            nc.sync.dma_start(h0f[:], h0fin[:])

    return _split_multi_waits(nc)


def _prep_core_inputs(c, x, encoder_h, W_ih, W_hh, b_ih, b_hh, t_steps):
    """Host-side layout prep for one core's batch shard."""
    bf = np.float16
    bsl = slice(c * BS, (c + 1) * BS)
    xs = x[bsl, :t_steps, :]                      # [BS, t, IN]
    # xT[p, k, t*BS+b] = x[b, t, 128k+p]
    xT = np.ascontiguousarray(
        xs.transpose(2, 1, 0).reshape(KT, 128, t_steps * BS).transpose(1, 0, 2)
    ).astype(bf)
    inmap = {"xT": xT}
    for l in range(L):
        # w[p, k, m] = W[l][m, 128k+p]
        inmap[f"wih{l}"] = np.ascontiguousarray(
            W_ih[l].T.reshape(KT, 128, H3).transpose(1, 0, 2)).astype(bf)
        inmap[f"whh{l}"] = np.ascontiguousarray(
            W_hh[l].T.reshape(KT, 128, H3).transpose(1, 0, 2)).astype(bf)
        bg = b_ih[l].astype(np.float32).copy()
        bg[:2 * H] += b_hh[l][:2 * H]             # fold r/z recurrent bias
        inmap[f"bgi{l}"] = np.ascontiguousarray(
            bg.reshape(MT, 128).T).astype(np.float32)
        inmap[f"bnw{l}"] = np.ascontiguousarray(
            b_hh[l][2 * H:].reshape(KT, 128)).astype(bf)
        inmap[f"ench{l}"] = np.ascontiguousarray(
            encoder_h[l, bsl, :].T.reshape(KT, 128, BS).transpose(1, 0, 2)
        ).astype(np.float32)
    inmap["sel"] = np.kron(np.eye(KT), np.ones((1, BS))).astype(bf)
    return inmap


_NC_CACHE = {}


def run_gru(x, encoder_h, W_ih, W_hh, b_ih, b_hh, t_steps=T_FULL, w_steps=64,
            trace=False):
    x = np.asarray(x, dtype=np.float32)
    encoder_h = np.asarray(encoder_h, dtype=np.float32)
    W_ih = np.asarray(W_ih, dtype=np.float32)
    W_hh = np.asarray(W_hh, dtype=np.float32)
    b_ih = np.asarray(b_ih, dtype=np.float32)
    b_hh = np.asarray(b_hh, dtype=np.float32)

    key = (t_steps, w_steps)
    if key not in _NC_CACHE:
        _NC_CACHE[key] = build_gru_nc(t_steps, w_steps)
    nc = _NC_CACHE[key]

    in_maps = [_prep_core_inputs(c, x, encoder_h, W_ih, W_hh, b_ih, b_hh, t_steps)
               for c in range(NCORES)]
    res = bass_utils.run_bass_kernel_spmd(
        nc, in_maps, core_ids=list(range(NCORES)), trace=trace)

    out = np.empty((B, t_steps, H), dtype=np.float32)
    h = np.empty((L, B, H), dtype=np.float32)
    for c in range(NCORES):
        o1 = np.asarray(res.results[c]["out1"])   # [128, KT, BS, t]
        # out[b, t, 128j+p] = o1[p, j, b, t]
        out[c * BS:(c + 1) * BS, :, :] = (
            o1.transpose(2, 3, 1, 0).reshape(BS, t_steps, H))
        h[1, c * BS:(c + 1) * BS, :] = (
            o1[:, :, :, t_steps - 1].transpose(2, 1, 0).reshape(BS, H))
        h0 = np.asarray(res.results[c]["h0f"])    # [128, KT, BS]
        h[0, c * BS:(c + 1) * BS, :] = h0.transpose(2, 1, 0).reshape(BS, H)
    return out, h, res


def kernel(x, encoder_h, W_ih, W_hh, b_ih, b_hh):
    out, h, _ = run_gru(x, encoder_h, W_ih, W_hh, b_ih, b_hh)
    return out, h


# revision 25
# speedup vs baseline: 1.0711x; 1.0711x over previous
"""Bass/Trainium2 kernel for a 2-layer GRU decoder.

Contract: kernel(**inputs) takes the FULL unsharded inputs (numpy arrays,
keys as in setup_inputs) and returns the full (out, h) pair, matching the
jax reference. Internally shards the batch across 8 NeuronCores.

Design (per core, Bs = 8 sequences):
  - hosts pre-transpose x and all weights so the device never transposes
  - per 64-step window: GI chunk matmuls (input-side gates, dense GEMM),
    then the two layer recurrences step-interleaved so each layer's gate
    (DVE/ACT) latency hides behind the other layer's 48 recurrent matmuls
  - weights stationary (bf16), h streamed as rhs; gates in [3H, Bs] layout
"""

import sys

sys.path.insert(0, "/opt/trn_rl_repo")

import numpy as np
import ml_dtypes

import concourse.bass as bass
import concourse.mybir as mybir
from concourse.tile import TileContext
from concourse.vector_clock import ScopedClock, VectorClock
from concourse.tile_scheduler import N_PROCS
from concourse.bass import ts
from concourse import bass_utils


def _install_ntff_hook_shim():
    """The container's antenv lacks axon_hooks; synthesize it so
    run_bass_kernel_spmd(trace=True) can NTFF-profile via the axon .so."""
    import types
    import antenv

    if "antenv.axon_hooks" in sys.modules:
        return
    mod = types.ModuleType("antenv.axon_hooks")
    _state = {"hook": None}
    mod.set_axon_ntff_profile_hook = lambda h: _state.__setitem__("hook", h)
    mod.get_axon_ntff_profile_hook = lambda: _state["hook"]
    sys.modules["antenv.axon_hooks"] = mod
    antenv.axon_hooks = mod
    try:
        from trn_agent_boot.trn_boot import _ntff_profile_via_ctypes
        mod.set_axon_ntff_profile_hook(
            _ntff_profile_via_ctypes("/opt/axon/libaxon_pjrt.so"))
    except Exception:
        pass


_install_ntff_hook_shim()

BF16 = mybir.dt.float16  # fp16: 10-bit mantissa, plenty of range here
F32 = mybir.dt.float32
AF = mybir.ActivationFunctionType

B, T_FULL, IN, H, L = 64, 512, 512, 512, 2
NCORES = 8
BS = B // NCORES          # 8 sequences per core
H3 = 3 * H                # 1536
KT = H // 128             # 4 contraction tiles
MT = H3 // 128            # 12 output tiles (r:0-3, z:4-7, n:8-11)


# --- workaround: this walrus build allows only one sem-wait per control
# instruction; split the TileContext exit drain into one drain per proc.
def _drain_and_barrier_split(self, tick_clock, wait_clock):
    g = tick_clock.global_clock
    for p in range(N_PROCS):
        if g[p] == 0:
            continue
        vec = [0] * N_PROCS
        vec[p] = g[p]
        d = self.nc.sync.drain()
        wait_clock.add_sem_waits(d.ins, ScopedClock({None: VectorClock(vec)}))
    self.nc.all_engine_barrier()
    assert self.sems is not None
    popped = self.nc._tile_sem_poison_stack.pop()
    assert popped is self._sem_poison
    self.nc.clear_and_free_semaphores(list(self.sems.allocated().values()))
    self.nc.all_engine_barrier()


TileContext._drain_and_barrier = _drain_and_barrier_split

_COMPUTE_ENGINES = {
    mybir.EngineType.PE,
    mybir.EngineType.Activation,
    mybir.EngineType.DVE,
    mybir.EngineType.Pool,
    mybir.EngineType.SP,
}


def _split_multi_waits(nc):
    """This walrus build allows one sem-wait per engine instruction; hoist
    extra waits onto single-wait NoOps inserted just before."""
    ctr = 0
    for f in nc.m.functions:
        for bb in f.blocks:
            il = bb.instructions
            i = 0
            while i < len(il):
                inst = il[i]
                si = inst.sync_info
                if (si is not None and len(si.on_wait) > 1
                        and inst.engine in _COMPUTE_ENGINES):
                    waits = list(si.on_wait)
                    nops = []
                    for w in waits[:-1]:
                        ctr += 1
                        nop = mybir.InstNoOp(
                            name=f"waitsplit-{ctr}", ins=[], outs=[])
                        nop.engine = inst.engine
                        nop.sync_info = mybir.SyncInfo(
                            on_wait=[w], on_update=[])
                        nops.append(nop)
                    inst.sync_info = mybir.SyncInfo(
                        on_wait=[waits[-1]], on_update=list(si.on_update))
                    il[i:i] = nops
                    i += len(nops)
                i += 1
    return nc


def build_gru_nc(t_steps: int, w_steps: int) -> bass.Bass:
    n_win = t_steps // w_steps
    assert n_win * w_steps == t_steps
    nw = w_steps * BS  # free-dim columns per window chunk

    nc = bass.Bass()

    # ---- DRAM I/O (host-prepped layouts) ----
    xT = nc.dram_tensor("xT", [128, KT, t_steps * BS], BF16, kind="ExternalInput")
    wih = [nc.dram_tensor(f"wih{l}", [128, KT, H3], BF16, kind="ExternalInput")
           for l in range(L)]
    whh = [nc.dram_tensor(f"whh{l}", [128, KT, H3], BF16, kind="ExternalInput")
           for l in range(L)]
    bgi = [nc.dram_tensor(f"bgi{l}", [128, MT], F32, kind="ExternalInput")
           for l in range(L)]
    # b_hh n-part as a [KT, 128] lhsT + one-hot selector: bias lands in PSUM
    # via a single K=4 matmul instead of a DVE add on the critical chain
    bnw = [nc.dram_tensor(f"bnw{l}", [KT, 128], BF16, kind="ExternalInput")
           for l in range(L)]
    sel = nc.dram_tensor("sel", [KT, KT * BS], BF16, kind="ExternalInput")
    ench = [nc.dram_tensor(f"ench{l}", [128, KT, BS], F32, kind="ExternalInput")
            for l in range(L)]
    # layer-1 hidden sequence (the decoder output), [p, j, b, t]
    out1 = nc.dram_tensor("out1", [128, KT, BS, t_steps], F32, kind="ExternalOutput")
    # layer-0 final hidden state
    h0f = nc.dram_tensor("h0f", [128, KT, BS], F32, kind="ExternalOutput")

    with TileContext(nc) as tc:
        with (
            tc.tile_pool(name="weights", bufs=1) as wpool,
            tc.tile_pool(name="state", bufs=1) as spool,
            tc.tile_pool(name="xchunk", bufs=1) as xpool,
            tc.tile_pool(name="tmp", bufs=4) as tpool,
            tc.tile_pool(name="gipsum", bufs=2, space="PSUM") as gps,
            tc.tile_pool(name="recpsum", bufs=1, space="PSUM") as rps,
        ):
            # ---- persistent SBUF tiles ----
            wih_sb = [wpool.tile([128, KT, H3], BF16, name=f"wih{l}") for l in range(L)]
            whh_sb = [wpool.tile([128, KT, H3], BF16, name=f"whh{l}") for l in range(L)]
            bgi_sb = [wpool.tile([128, MT], F32, name=f"bgi{l}") for l in range(L)]
            bnw_sb = [wpool.tile([KT, 128], BF16, name=f"bnw{l}") for l in range(L)]
            sel_sb = wpool.tile([KT, KT * BS], BF16, name="sel_sb")
            # GI double-buffer slots, [p, m, i, b]
            gi0_sl = [spool.tile([128, MT, w_steps, BS], BF16, name=f"gi0_{s}")
                      for s in range(2)]
            gi1_sl = [spool.tile([128, MT, w_steps, BS], BF16, name=f"gi1_{s}")
                      for s in range(2)]
            # hidden-state rings, [p, k, pos, b]; pos 0 = window-entry state
            h0_sl = [spool.tile([128, KT, w_steps + 1, BS], BF16, name=f"h0_{s}")
                     for s in range(2)]
            h1_sl = [spool.tile([128, KT, w_steps + 1, BS], BF16, name=f"h1_{s}")
                     for s in range(2)]
            # fp32 layer-1 output staging, [p, j, b, i] (t innermost for DMA)
            ho_sl = [spool.tile([128, KT, BS, w_steps], F32, name=f"ho_{s}")
                     for s in range(2)]
            xt_sl = [xpool.tile([128, KT, nw], BF16, name=f"xt_{s}")
                     for s in range(2)]
            # recurrence psum, split rz|n into separate banks so the rz
            # gates can start while the n-tiles are still multiplying
            ps_rz = [rps.tile([128, 8, BS], F32, name=f"psrz{l}", tag=f"psrz{l}")
                     for l in range(L)]
            ps_n = [rps.tile([128, KT, BS], F32, name=f"psn{l}", tag=f"psn{l}")
                    for l in range(L)]

            # ---- prologue: weights, biases, initial hidden states ----
            for l in range(L):
                nc.sync.dma_start(wih_sb[l][:], wih[l][:])
                nc.sync.dma_start(whh_sb[l][:], whh[l][:])
                nc.sync.dma_start(bgi_sb[l][:], bgi[l][:])
                nc.sync.dma_start(bnw_sb[l][:], bnw[l][:])
            nc.sync.dma_start(sel_sb[:], sel[:])
            ench_sb = [tpool.tile([128, KT, BS], F32, tag=f"ench{l}",
                                  name=f"ench_sb{l}") for l in range(L)]
            for l in range(L):
                nc.sync.dma_start(ench_sb[l][:], ench[l][:])
            nc.vector.tensor_copy(h0_sl[0][:, :, 0, :], ench_sb[0][:])
            nc.vector.tensor_copy(h1_sl[0][:, :, 0, :], ench_sb[1][:])

            SUB = 8  # GI1 sub-chunk length in steps

            def gi0_microops(wdst):
                """Per-instruction closures for GI0 of window wdst (from x)."""
                sl = wdst % 2
                ops = []
                for m in range(MT):
                    pg = gps.tile([128, w_steps, BS], F32, tag="gip0",
                                  name=f"gip0_{wdst}_{m}")
                    for k in range(KT):
                        ops.append(lambda m=m, k=k, pg=pg: nc.tensor.matmul(
                            pg[:], lhsT=wih_sb[0][:, k, ts(m, 128)],
                            rhs=xt_sl[sl][:, k, :],
                            start=(k == 0), stop=(k == KT - 1)))
                    # copy on DVE, not ACT: an ACT-queue copy waiting on its
                    # matmuls head-of-line-blocks the gate sigmoids/tanhs
                    ops.append(lambda m=m, pg=pg: nc.vector.tensor_scalar_add(
                        gi0_sl[sl][:, m], pg[:], bgi_sb[0][:, m:m + 1]))
                return ops

            def gi1_subchunk_ops(w, q):
                """m-tile closures for GI1 window w, steps [SUB*q, SUB*(q+1))."""
                sl = w % 2
                ops = []
                for m in range(MT):
                    def one(m=m, q=q, sl=sl):
                        pg = gps.tile([128, SUB, BS], F32, tag="gip1",
                                      name=f"gip1_{w}_{q}_{m}")
                        for k in range(KT):
                            nc.tensor.matmul(
                                pg[:], lhsT=wih_sb[1][:, k, ts(m, 128)],
                                rhs=h1rhs(sl, k, q),
                                start=(k == 0), stop=(k == KT - 1))
                        nc.vector.tensor_scalar_add(
                            gi1_sl[sl][:, m, ts(q, SUB), :], pg[:],
                            bgi_sb[1][:, m:m + 1])
                    ops.append(one)
                return ops

            def h1rhs(sl, k, q):
                return h0_sl[sl][:, k, 1 + SUB * q: 1 + SUB * (q + 1), :]

            def step(l, h_slot, i, gi_slot, ho_slot):
                """One GRU cell step: h[pos i] -> h[pos i+1]."""
                prz, pn = ps_rz[l], ps_n[l]
                # r/z recurrent matmuls first -> their own psum bank
                for m in range(8):
                    for k in range(KT):
                        nc.tensor.matmul(
                            prz[:, m, :],
                            lhsT=whh_sb[l][:, k, ts(m, 128)],
                            rhs=h_slot[:, k, i, :],
                            start=(k == 0), stop=(k == KT - 1))
                # n-part: bias lands via one K=KT selector matmul, then accum
                nc.tensor.matmul(pn[:], lhsT=bnw_sb[l][:], rhs=sel_sb[:],
                                 start=True, stop=False, skip_group_check=True)
                for m in range(KT):
                    for k in range(KT):
                        nc.tensor.matmul(
                            pn[:, m, :],
                            lhsT=whh_sb[l][:, k, ts(8 + m, 128)],
                            rhs=h_slot[:, k, i, :],
                            start=False, stop=(k == KT - 1),
                            skip_group_check=True)
                gi_rz = gi_slot[:, 0:8, i, :]
                gi_n = gi_slot[:, 8:12, i, :]
                h_prev = h_slot[:, :, i, :]
                h_out = h_slot[:, :, i + 1, :]

                trz = tpool.tile([128, 8, BS], F32, tag="trz")
                nc.vector.tensor_add(trz[:], prz[:, 0:8, :], gi_rz)
                rzb = tpool.tile([128, 8, BS], BF16, tag="rzb")
                nc.scalar.activation(rzb[:], trz[:], AF.Sigmoid)
                # t1 = z*h_prev runs in parallel with the tanh path
                t1 = tpool.tile([128, KT, BS], BF16, tag="t1")
                nc.vector.tensor_mul(t1[:], rzb[:, 4:8, :], h_prev)
                u = tpool.tile([128, KT, BS], F32, tag="u")
                nc.vector.tensor_mul(u[:], pn[:], rzb[:, 0:4, :])
                wt = tpool.tile([128, KT, BS], F32, tag="wt")
                nc.vector.tensor_add(wt[:], u[:], gi_n)
                nt = tpool.tile([128, KT, BS], BF16, tag="nt")
                nc.scalar.activation(nt[:], wt[:], AF.Tanh)
                # a = (z-1)*n ; h' = z*h - a = z*h + (1-z)*n
                a = tpool.tile([128, KT, BS], BF16, tag="a")
                nc.vector.scalar_tensor_tensor(
                    a[:], rzb[:, 4:8, :], 1.0, nt[:],
                    op0=mybir.AluOpType.subtract, op1=mybir.AluOpType.mult)
                nc.vector.tensor_sub(h_out, t1[:], a[:])
                if ho_slot is not None:
                    # fp32 copy of h for the output stream (off the chain)
                    nc.gpsimd.tensor_sub(ho_slot[:, :, :, i], t1[:], a[:])

            # ---- main loop: window w runs L0 on [64w, 64w+64) and L1 one
            # window behind; GI0(w+1) and GI1(w) matmuls are spread between
            # steps to keep the PE busy (and HAM-warm) during gate latency.
            nc.sync.dma_start(xt_sl[0][:], xT[:, :, ts(0, nw)])
            for op in gi0_microops(0):
                op()
            for w in range(n_win + 1):
                s = w % 2
                sp = (w - 1) % 2
                gi0_q = []
                if w + 1 < n_win:
                    nc.sync.dma_start(xt_sl[(w + 1) % 2][:],
                                      xT[:, :, ts(w + 1, nw)])
                    gi0_q = gi0_microops(w + 1)
                gi1_q = []
                for i in range(w_steps):
                    if w < n_win:
                        step(0, h0_sl[s], i, gi0_sl[s], None)
                        if i % SUB == SUB - 1 and i != w_steps - 1:
                            gi1_q.extend(gi1_subchunk_ops(w, i // SUB))
                    if w >= 1:
                        step(1, h1_sl[sp], i, gi1_sl[sp], ho_sl[sp])
                    # filler: keep the PE streaming between steps
                    if gi0_q:
                        gi0_q.pop(0)()
                    for _ in range(2):
                        if gi1_q:
                            gi1_q.pop(0)()
                if w < n_win:
                    gi1_q.extend(gi1_subchunk_ops(w, w_steps // SUB - 1))
                    for op in gi1_q:
                        op()
                    for op in gi0_q:
                        op()
                    # carry window-exit state into the next ring slot
                    if w + 1 < n_win:
                        nc.vector.tensor_copy(h0_sl[(w + 1) % 2][:, :, 0, :],
                                              h0_sl[s][:, :, w_steps, :])
                if w >= 1:
                    nc.sync.dma_start(out1[:, :, :, ts(w - 1, w_steps)],
                                      ho_sl[sp][:])
                    if w < n_win:
                        nc.vector.tensor_copy(h1_sl[w % 2][:, :, 0, :],
                                              h1_sl[sp][:, :, w_steps, :])

            # layer-0 final hidden state -> fp32 -> DRAM
            h0fin = tpool.tile([128, KT, BS], F32, tag="h0fin")
            nc.vector.tensor_copy(h0fin[:], h0_sl[(n_win - 1) % 2][:, :, w_steps, :])
            nc.sync.dma_start(h0f[:], h0fin[:])

    return _split_multi_waits(nc)


def _prep_core_inputs(c, x, encoder_h, W_ih, W_hh, b_ih, b_hh, t_steps):
    """Host-side layout prep for one core's batch shard."""
    bf = np.float16
    bsl = slice(c * BS, (c + 1) * BS)
    xs = x[bsl, :t_steps, :]                      # [BS, t, IN]
    # xT[p, k, t*BS+b] = x[b, t, 128k+p]
    xT = np.ascontiguousarray(
        xs.transpose(2, 1, 0).reshape(KT, 128, t_steps * BS).transpose(1, 0, 2)
    ).astype(bf)
    inmap = {"xT": xT}
    for l in range(L):
        # w[p, k, m] = W[l][m, 128k+p]
        inmap[f"wih{l}"] = np.ascontiguousarray(
            W_ih[l].T.reshape(KT, 128, H3).transpose(1, 0, 2)).astype(bf)
        inmap[f"whh{l}"] = np.ascontiguousarray(
            W_hh[l].T.reshape(KT, 128, H3).transpose(1, 0, 2)).astype(bf)
        bg = b_ih[l].astype(np.float32).copy()
        bg[:2 * H] += b_hh[l][:2 * H]             # fold r/z recurrent bias
        inmap[f"bgi{l}"] = np.ascontiguousarray(
            bg.reshape(MT, 128).T).astype(np.float32)
        inmap[f"bnw{l}"] = np.ascontiguousarray(
            b_hh[l][2 * H:].reshape(KT, 128)).astype(bf)
        inmap[f"ench{l}"] = np.ascontiguousarray(
            encoder_h[l, bsl, :].T.reshape(KT, 128, BS).transpose(1, 0, 2)
        ).astype(np.float32)
    inmap["sel"] = np.kron(np.eye(KT), np.ones((1, BS))).astype(bf)
    return inmap


_NC_CACHE = {}


def run_gru(x, encoder_h, W_ih, W_hh, b_ih, b_hh, t_steps=T_FULL, w_steps=64,
            trace=False):
    x = np.asarray(x, dtype=np.float32)
    encoder_h = np.asarray(encoder_h, dtype=np.float32)
    W_ih = np.asarray(W_ih, dtype=np.float32)
    W_hh = np.asarray(W_hh, dtype=np.float32)
    b_ih = np.asarray(b_ih, dtype=np.float32)
    b_hh = np.asarray(b_hh, dtype=np.float32)

    key = (t_steps, w_steps)
    if key not in _NC_CACHE:
        _NC_CACHE[key] = build_gru_nc(t_steps, w_steps)
    nc = _NC_CACHE[key]

    in_maps = [_prep_core_inputs(c, x, encoder_h, W_ih, W_hh, b_ih, b_hh, t_steps)
               for c in range(NCORES)]
    res = bass_utils.run_bass_kernel_spmd(
        nc, in_maps, core_ids=list(range(NCORES)), trace=trace)

    out = np.empty((B, t_steps, H), dtype=np.float32)
    h = np.empty((L, B, H), dtype=np.float32)
    for c in range(NCORES):
        o1 = np.asarray(res.results[c]["out1"])   # [128, KT, BS, t]
        # out[b, t, 128j+p] = o1[p, j, b, t]
        out[c * BS:(c + 1) * BS, :, :] = (
            o1.transpose(2, 3, 1, 0).reshape(BS, t_steps, H))
        h[1, c * BS:(c + 1) * BS, :] = (
            o1[:, :, :, t_steps - 1].transpose(2, 1, 0).reshape(BS, H))
        h0 = np.asarray(res.results[c]["h0f"])    # [128, KT, BS]
        h[0, c * BS:(c + 1) * BS, :] = h0.transpose(2, 1, 0).reshape(BS, H)
    return out, h, res


def kernel(x, encoder_h, W_ih, W_hh, b_ih, b_hh):
    out, h, _ = run_gru(x, encoder_h, W_ih, W_hh, b_ih, b_hh)
    return out, h
